# revision 19
# baseline (speedup 1.0000x reference)
"""Trainium2 Bass kernel for nn_RNN_87351044866483.

Reference semantics: every timestep is independent (no recurrence carried
across t) and only the last timestep feeds the readout, so the output
depends only on X[:, -1]:

    x   = emb[X[:, -1]]                  # [B, E]
    h   = tanh(x @ W_xh + b_h)           # [B, H]
    h   = tanh(h @ W_hh + b_h)   (x4)
    out = h @ W_hy + b_y                 # [B, V]

Sharding (8 cores):
  * h-chain is tensor-parallel: W_xh / W_hh / b_h are column-sharded
    (256 features per core); each core computes its 2 m-tiles of the next
    h and an AllGather rebuilds the full h between layers. This matters
    because with B=64 the PE is LDWEIGHTS-bound (fp32 weights load at
    ~426ns per 128x128 tile vs ~107ns of streaming), so replicating the
    h-chain costs 8x the weight-load time.
  * W_hy / b_y are column-sharded (V/8 = 4000 per core); each core emits
    its own output slice, gathered on the host.

On-chip layout: activations are feature-major ("hT layout"):
hT_sb[p, m*64 + b] = h[b, m*128 + p]. Core c owns m-tiles {2c, 2c+1}, so
rank-order AllGather output is exactly the next hT layout (512B segments).
The h-chain stays full fp32 (the tanh layers amplify injected error ~5x
per layer); the readout uses float32r single-pass matmuls (error lands
below the fp32 summation-order noise floor of this problem).
"""

import sys
import types

import numpy as np

B = 64
E = 1024
H = 2048
V = 32000
N_CORES = 8
V_SH = V // N_CORES       # 4000
H_SH = H // N_CORES       # 256 features per core
ML = H_SH // 128          # 2 local m-tiles
NUM_LAYERS = 4

EK = E // 128   # 8  k-tiles in layer 0
HK = H // 128   # 16 k-tiles in hidden layers
HM = H // 128   # 16 m-tiles of hidden features
NC_CHUNK = 2000           # W_hy columns per streamed tile (1 MB DMA)
NSUB = 500                # readout psum free dim (<=512 fp32)
WHY_BUFS = 16             # one full half of W_hy resident (prefetched)

READOUT_F32R = True

_cache = {}


def _ensure_axon_hooks():
    """antenv.axon_hooks is absent in this image; recreate it so
    run_bass_kernel_spmd(trace=True) can import it (used by test.py)."""
    if "antenv.axon_hooks" in sys.modules:
        return
    m = types.ModuleType("antenv.axon_hooks")
    m._hook = None
    m.set_axon_ntff_profile_hook = lambda h: setattr(m, "_hook", h)
    m.get_axon_ntff_profile_hook = lambda: m._hook
    sys.modules["antenv.axon_hooks"] = m
    try:
        import antenv
        antenv.axon_hooks = m
        from trn_agent_boot.trn_boot import _ntff_profile_via_ctypes
        hook = _ntff_profile_via_ctypes("/opt/axon/libaxon_pjrt.so")
        m.set_axon_ntff_profile_hook(hook)
    except Exception:
        pass


def _build_nc():
    import concourse.bacc as bacc
    import concourse.mybir as mybir
    import concourse.tile as tile

    f32 = mybir.dt.float32
    f32r = mybir.dt.float32r
    Tanh = mybir.ActivationFunctionType.Tanh

    nc = bacc.Bacc(None, target_bir_lowering=False, num_devices=N_CORES)

    xt_d = nc.dram_tensor("xt", [128, EK * B], f32, kind="ExternalInput")
    wxh_d = nc.dram_tensor("wxh", [E, H_SH], f32, kind="ExternalInput")
    whh_d = nc.dram_tensor("whh", [H, H_SH], f32, kind="ExternalInput")
    why_d = nc.dram_tensor("why", [H, V_SH],
                           f32r if READOUT_F32R else f32, kind="ExternalInput")
    bh_d = nc.dram_tensor("bh", [128, ML], f32, kind="ExternalInput")
    out_d = nc.dram_tensor("out", [B, V_SH], f32, kind="ExternalOutput")

    # Per-layer collective staging buffers (DRAM). Shared address space is
    # required for the AllGather output.
    n_dense = 1 + NUM_LAYERS
    cc_in = [nc.dram_tensor(f"cc_in{l}", [128, H_SH // 2], f32)
             for l in range(n_dense)]
    cc_out = [nc.dram_tensor(f"cc_out{l}", [N_CORES, 128, H_SH // 2], f32,
                             addr_space="Shared")
              for l in range(n_dense)]

    groups = [list(range(N_CORES))]

    with tile.TileContext(nc) as tc:
        with tc.tile_pool(name="const", bufs=1) as const_pool, \
             tc.tile_pool(name="wsb", bufs=1) as w_pool, \
             tc.tile_pool(name="hg", bufs=2) as hg_pool, \
             tc.tile_pool(name="hs", bufs=2) as hs_pool, \
             tc.tile_pool(name="lpsum", bufs=2, space="PSUM") as lpsum:

            bh_t = const_pool.tile([128, ML], f32, tag="bh")
            nc.sync.dma_start(out=bh_t[:], in_=bh_d[:])
            xt_t = const_pool.tile([128, EK * B], f32, tag="xt")
            nc.sync.dma_start(out=xt_t[:], in_=xt_d[:])

            # Column-sharded weights, repacked so k-tile k / local column j
            # sits at sbuf[:, k*H_SH + j] (single large DMA each).
            wxh_t = w_pool.tile([128, EK * H_SH], f32, tag="wxh")
            nc.gpsimd.dma_start(
                out=wxh_t[:].rearrange("p (k j) -> p k j", k=EK),
                in_=wxh_d.rearrange("(k p) j -> p k j", p=128))
            whh_t = w_pool.tile([128, HK * H_SH], f32, tag="whh")
            nc.gpsimd.dma_start(
                out=whh_t[:].rearrange("p (k j) -> p k j", k=HK),
                in_=whh_d.rearrange("(k p) j -> p k j", p=128))

            mm_dt = f32r if READOUT_F32R else f32
            why_pool = tc.alloc_tile_pool(name="why", bufs=WHY_BUFS)

            def why_dma(half):
                tiles = []
                for k in range(HK):
                    wt = why_pool.tile([128, NC_CHUNK], mm_dt, tag="why",
                                       name=f"why{half}_{k}")
                    nc.sync.dma_start(
                        out=wt[:],
                        in_=why_d[k * 128:(k + 1) * 128,
                                  half * NC_CHUNK:(half + 1) * NC_CHUNK])
                    tiles.append(wt)
                return tiles

            # Prefetch half 0 of W_hy now: these DMAs have no waits, so they
            # enqueue to the SDMA engines behind the wxh/whh loads and stream
            # during the h-chain (the AllGather triggers emitted below would
            # otherwise head-of-line-block the gpsimd queue).
            why_half0 = why_dma(0)

            # PE warm-up: junk matmuls into a scratch psum bank keep the HAM
            # clock at 2.4 GHz across the AllGather gaps (PE-idle > ~3.4us
            # re-throttles to 1.2 GHz, doubling every matmul afterwards).
            warm_ps = lpsum.tile([128, B], f32, tag="warm", name="warm_ps")

            def pe_warmup(n, tag):
                for w in range(n):
                    nc.tensor.matmul(
                        warm_ps[:, :], wxh_t[:, 0:128], xt_t[:, 0:B],
                        start=True, stop=True)

            def dense_layer(w_t, rhs_t, nk, lidx, out_f32r=False):
                """Local m-shard matmul + tanh, then AllGather into the full
                hT layout. Returns the gathered [128, HM*B] tile."""
                ps = lpsum.tile([128, ML * B], f32, tag="lp", name=f"lp{lidx}")
                h_shard = hs_pool.tile([128, ML * B], f32, tag="hs",
                                       name=f"hs{lidx}")
                for k in range(nk):
                    for i in range(ML):
                        nc.tensor.matmul(
                            ps[:, i * B:(i + 1) * B],
                            w_t[:, k * H_SH + i * 128: k * H_SH + (i + 1) * 128],
                            rhs_t[:, k * B:(k + 1) * B],
                            start=(k == 0 and i == 0),
                            stop=(k == nk - 1 and i == ML - 1))
                for i in range(ML):
                    nc.scalar.activation(
                        h_shard[:, i * B:(i + 1) * B],
                        ps[:, i * B:(i + 1) * B], Tanh,
                        bias=bh_t[:, i:i + 1])
                nc.scalar.dma_start(out=cc_in[lidx][:], in_=h_shard[:])
                nc.gpsimd.collective_compute(
                    "AllGather", mybir.AluOpType.bypass,
                    replica_groups=groups,
                    ins=[cc_in[lidx][:]],
                    outs=[cc_out[lidx][:]])
                if out_f32r:
                    # Casting gather (SWDGE): lands pre-rounded f32r for the
                    # readout's stationary operand, skipping a DVE cast.
                    h_full = hg_pool.tile([128, HM * B], f32r, tag="hg",
                                          name=f"hf{lidx}")
                    nc.gpsimd.dma_start(
                        out=h_full[:].rearrange("p (c q) -> p c q", c=N_CORES),
                        in_=cc_out[lidx].rearrange("c p q -> p c q"))
                else:
                    h_full = hg_pool.tile([128, HM * B], f32, tag="hg",
                                          name=f"hf{lidx}")
                    nc.scalar.dma_start(
                        out=h_full[:].rearrange("p (c q) -> p c q", c=N_CORES),
                        in_=cc_out[lidx].rearrange("c p q -> p c q"))
                return h_full

            h_cur = dense_layer(wxh_t, xt_t, EK, 0)
            pe_warmup(40, "w0")
            for layer in range(NUM_LAYERS):
                last = layer == NUM_LAYERS - 1
                h_cur = dense_layer(whh_t, h_cur, HK, layer + 1,
                                    out_f32r=(READOUT_F32R and last))
                pe_warmup(80 if last else 40, f"w{layer + 1}")

            # ---- readout: out = h @ Why (b_y added on host) ----
            n_halves = V_SH // NC_CHUNK          # 2
            n_sub = NC_CHUNK // NSUB             # 4
            with tc.tile_pool(name="outsb", bufs=4) as out_pool, \
                 tc.tile_pool(name="rpsum", bufs=4, space="PSUM") as rpsum:
                h_r = h_cur
                for half in range(n_halves):
                    why_tiles = why_half0 if half == 0 else why_dma(1)

                    ps_list = [rpsum.tile([B, NSUB], f32, tag="rp",
                                          name=f"rp{half}_{c}")
                               for c in range(n_sub)]
                    for k in range(HK):
                        for c in range(n_sub):
                            nc.tensor.matmul(
                                ps_list[c][:, :],
                                h_r[:, k * B:(k + 1) * B],
                                why_tiles[k][:, c * NSUB:(c + 1) * NSUB],
                                start=(k == 0), stop=(k == HK - 1))
                    for c in range(n_sub):
                        ot = out_pool.tile([B, NSUB], f32, tag="o",
                                           name=f"o{half}_{c}")
                        nc.vector.tensor_copy(ot[:], ps_list[c][:])
                        off = half * NC_CHUNK + c * NSUB
                        nc.gpsimd.dma_start(out=out_d[:, off:off + NSUB], in_=ot[:])
            why_pool.release()

    nc.compile()
    return nc


def _get_nc():
    if "nc" not in _cache:
        _ensure_axon_hooks()
        _cache["nc"] = _build_nc()
    return _cache["nc"]


def make_in_maps(X, emb, W_xh, W_hh, W_hy, b_h, b_y):
    X = np.asarray(X)
    emb = np.asarray(emb, dtype=np.float32)
    W_xh = np.asarray(W_xh, dtype=np.float32)
    W_hh = np.asarray(W_hh, dtype=np.float32)
    W_hy = np.asarray(W_hy, dtype=np.float32)
    b_h = np.asarray(b_h, dtype=np.float32)

    idx = np.asarray(X[:, -1], dtype=np.int64)
    x = emb[idx]                                    # [B, E]
    # xt_tiled[p, k*B + b] = x[b, k*128 + p]
    xt = np.ascontiguousarray(
        x.reshape(B, EK, 128).transpose(2, 1, 0).reshape(128, EK * B))

    in_maps = []
    for c in range(N_CORES):
        sl = slice(c * H_SH, (c + 1) * H_SH)
        in_maps.append({
            "xt": xt,
            "wxh": np.ascontiguousarray(W_xh[:, sl]),
            "whh": np.ascontiguousarray(W_hh[:, sl]),
            "why": np.ascontiguousarray(W_hy[:, c * V_SH:(c + 1) * V_SH]),
            # bh_sh[p, i] = b_h[c*256 + i*128 + p]
            "bh": np.ascontiguousarray(b_h[sl].reshape(ML, 128).T),
        })
    return in_maps


def kernel(X, emb, W_xh, W_hh, W_hy, b_h, b_y):
    from concourse.bass_utils import run_bass_kernel_spmd

    nc = _get_nc()
    in_maps = make_in_maps(X, emb, W_xh, W_hh, W_hy, b_h, b_y)
    res = run_bass_kernel_spmd(nc, in_maps, core_ids=list(range(N_CORES)))
    out = np.concatenate([res.results[c]["out"] for c in range(N_CORES)], axis=1)
    out = out + np.asarray(b_y, dtype=np.float32)[None, :]
    return out.astype(np.float32)


# revision 23
# speedup vs baseline: 1.0608x; 1.0608x over previous
"""Trainium2 Bass kernel for nn_RNN_87351044866483.

Reference semantics: every timestep is independent (no recurrence carried
across t) and only the last timestep feeds the readout, so the output
depends only on X[:, -1]:

    x   = emb[X[:, -1]]                  # [B, E]
    h   = tanh(x @ W_xh + b_h)           # [B, H]
    h   = tanh(h @ W_hh + b_h)   (x4)
    out = h @ W_hy + b_y                 # [B, V]

Sharding (8 cores):
  * h-chain is tensor-parallel: W_xh / W_hh / b_h are column-sharded
    (256 features per core); each core computes its 2 m-tiles of the next
    h and an AllGather rebuilds the full h between layers. This matters
    because with B=64 the PE is LDWEIGHTS-bound (fp32 weights load at
    ~426ns per 128x128 tile vs ~107ns of streaming), so replicating the
    h-chain costs 8x the weight-load time.
  * W_hy / b_y are column-sharded (V/8 = 4000 per core); each core emits
    its own output slice, gathered on the host.

On-chip layout: activations are feature-major ("hT layout"):
hT_sb[p, m*64 + b] = h[b, m*128 + p]. Core c owns m-tiles {2c, 2c+1}, so
rank-order AllGather output is exactly the next hT layout (512B segments).
The h-chain stays full fp32 (the tanh layers amplify injected error ~5x
per layer); the readout uses float32r single-pass matmuls (error lands
below the fp32 summation-order noise floor of this problem).
"""

import sys
import types

import numpy as np

B = 64
E = 1024
H = 2048
V = 32000
N_CORES = 8
V_SH = V // N_CORES       # 4000
H_SH = H // N_CORES       # 256 features per core
ML = H_SH // 128          # 2 local m-tiles
NUM_LAYERS = 4

EK = E // 128   # 8  k-tiles in layer 0
HK = H // 128   # 16 k-tiles in hidden layers
HM = H // 128   # 16 m-tiles of hidden features
NC_CHUNK = 2000           # W_hy columns per streamed tile (1 MB DMA)
NSUB = 500                # readout psum free dim (<=512 fp32)
WHY_BUFS = 16             # one full half of W_hy resident (prefetched)

READOUT_F32R = True

_cache = {}


def _ensure_axon_hooks():
    """antenv.axon_hooks is absent in this image; recreate it so
    run_bass_kernel_spmd(trace=True) can import it (used by test.py)."""
    if "antenv.axon_hooks" in sys.modules:
        return
    m = types.ModuleType("antenv.axon_hooks")
    m._hook = None
    m.set_axon_ntff_profile_hook = lambda h: setattr(m, "_hook", h)
    m.get_axon_ntff_profile_hook = lambda: m._hook
    sys.modules["antenv.axon_hooks"] = m
    try:
        import antenv
        antenv.axon_hooks = m
        from trn_agent_boot.trn_boot import _ntff_profile_via_ctypes
        hook = _ntff_profile_via_ctypes("/opt/axon/libaxon_pjrt.so")
        m.set_axon_ntff_profile_hook(hook)
    except Exception:
        pass


def _build_nc():
    import concourse.bacc as bacc
    import concourse.mybir as mybir
    import concourse.tile as tile
    from concourse.tile import add_dep_helper

    f32 = mybir.dt.float32
    f32r = mybir.dt.float32r
    Tanh = mybir.ActivationFunctionType.Tanh

    nc = bacc.Bacc(None, target_bir_lowering=False, num_devices=N_CORES)

    xt_d = nc.dram_tensor("xt", [128, EK * B], f32, kind="ExternalInput")
    wxh_d = nc.dram_tensor("wxh", [E, H_SH], f32, kind="ExternalInput")
    whh_d = nc.dram_tensor("whh", [H, H_SH], f32, kind="ExternalInput")
    why_d = nc.dram_tensor("why", [H, V_SH],
                           f32r if READOUT_F32R else f32, kind="ExternalInput")
    bh_d = nc.dram_tensor("bh", [128, ML], f32, kind="ExternalInput")
    out_d = nc.dram_tensor("out", [B, V_SH], f32, kind="ExternalOutput")

    # Per-layer collective staging buffers (DRAM). Shared address space is
    # required for the AllGather output.
    n_dense = 1 + NUM_LAYERS
    cc_in = [nc.dram_tensor(f"cc_in{l}", [128, H_SH // 2], f32)
             for l in range(n_dense)]
    cc_out = [nc.dram_tensor(f"cc_out{l}", [N_CORES, 128, H_SH // 2], f32,
                             addr_space="Shared")
              for l in range(n_dense)]

    groups = [list(range(N_CORES))]

    with tile.TileContext(nc) as tc:
        with tc.tile_pool(name="const", bufs=1) as const_pool, \
             tc.tile_pool(name="wsb", bufs=1) as w_pool, \
             tc.tile_pool(name="hg", bufs=2) as hg_pool, \
             tc.tile_pool(name="hs", bufs=2) as hs_pool, \
             tc.tile_pool(name="lpsum", bufs=2, space="PSUM") as lpsum:

            bh_t = const_pool.tile([128, ML], f32, tag="bh")
            nc.sync.dma_start(out=bh_t[:], in_=bh_d[:])
            xt_t = const_pool.tile([128, EK * B], f32, tag="xt")
            nc.sync.dma_start(out=xt_t[:], in_=xt_d[:])

            # Column-sharded weights, repacked so k-tile k / local column j
            # sits at sbuf[:, k*H_SH + j] (single large DMA each).
            wxh_t = w_pool.tile([128, EK * H_SH], f32, tag="wxh")
            nc.gpsimd.dma_start(
                out=wxh_t[:].rearrange("p (k j) -> p k j", k=EK),
                in_=wxh_d.rearrange("(k p) j -> p k j", p=128))
            whh_t = w_pool.tile([128, HK * H_SH], f32, tag="whh")
            whh_dma = nc.gpsimd.dma_start(
                out=whh_t[:].rearrange("p (k j) -> p k j", k=HK),
                in_=whh_d.rearrange("(k p) j -> p k j", p=128))

            mm_dt = f32r if READOUT_F32R else f32
            why_pool = tc.alloc_tile_pool(name="why", bufs=WHY_BUFS)

            def why_dma(half):
                tiles = []
                for k in range(HK):
                    wt = why_pool.tile([128, NC_CHUNK], mm_dt, tag="why",
                                       name=f"why{half}_{k}")
                    d = nc.sync.dma_start(
                        out=wt[:],
                        in_=why_d[k * 128:(k + 1) * 128,
                                  half * NC_CHUNK:(half + 1) * NC_CHUNK])
                    if half == 0 and k == 0:
                        add_dep_helper(d.ins, whh_dma.ins, sync=True,
                                       reason="why stream after h-chain weights")
                    tiles.append(wt)
                return tiles

            # Prefetch half 0 of W_hy on the sync queue (nothing is emitted
            # behind it there, so ring-full stalls are harmless); the first
            # transfer waits for the h-chain weights so layer 0 is not
            # bandwidth-starved at startup.
            why_half0 = why_dma(0)

            # PE warm-up: junk matmuls into a scratch psum bank keep the HAM
            # clock at 2.4 GHz across the AllGather gaps (PE-idle > ~3.4us
            # re-throttles to 1.2 GHz, doubling every matmul afterwards).
            # Each batch reads the layer's h_shard so the scheduler cannot
            # hoist it ahead of that layer (hoisted junk serializes in front
            # of real work on the PE).
            warm_ps = lpsum.tile([128, B], f32, tag="warm", name="warm_ps")

            def pe_warmup(n, dep_t):
                for w in range(n):
                    nc.tensor.matmul(
                        warm_ps[:, :], wxh_t[:, 0:128], dep_t[:, 0:B],
                        start=True, stop=True)

            def dense_layer(w_t, rhs_t, nk, lidx, out_f32r=False):
                """Local m-shard matmul + tanh, then AllGather into the full
                hT layout. Returns the gathered [128, HM*B] tile."""
                ps = lpsum.tile([128, ML * B], f32, tag="lp", name=f"lp{lidx}")
                h_shard = hs_pool.tile([128, ML * B], f32, tag="hs",
                                       name=f"hs{lidx}")
                h_shards.append(h_shard)
                for k in range(nk):
                    for i in range(ML):
                        nc.tensor.matmul(
                            ps[:, i * B:(i + 1) * B],
                            w_t[:, k * H_SH + i * 128: k * H_SH + (i + 1) * 128],
                            rhs_t[:, k * B:(k + 1) * B],
                            start=(k == 0 and i == 0),
                            stop=(k == nk - 1 and i == ML - 1))
                for i in range(ML):
                    nc.scalar.activation(
                        h_shard[:, i * B:(i + 1) * B],
                        ps[:, i * B:(i + 1) * B], Tanh,
                        bias=bh_t[:, i:i + 1])
                nc.scalar.dma_start(out=cc_in[lidx][:], in_=h_shard[:])
                nc.gpsimd.collective_compute(
                    "AllGather", mybir.AluOpType.bypass,
                    replica_groups=groups,
                    ins=[cc_in[lidx][:]],
                    outs=[cc_out[lidx][:]])
                if out_f32r:
                    # Casting gather (SWDGE): lands pre-rounded f32r for the
                    # readout's stationary operand, skipping a DVE cast.
                    h_full = hg_pool.tile([128, HM * B], f32r, tag="hg",
                                          name=f"hf{lidx}")
                    nc.gpsimd.dma_start(
                        out=h_full[:].rearrange("p (c q) -> p c q", c=N_CORES),
                        in_=cc_out[lidx].rearrange("c p q -> p c q"))
                else:
                    h_full = hg_pool.tile([128, HM * B], f32, tag="hg",
                                          name=f"hf{lidx}")
                    nc.scalar.dma_start(
                        out=h_full[:].rearrange("p (c q) -> p c q", c=N_CORES),
                        in_=cc_out[lidx].rearrange("c p q -> p c q"))
                return h_full

            h_shards = []
            h_cur = dense_layer(wxh_t, xt_t, EK, 0)
            pe_warmup(20, h_shards[-1])
            for layer in range(NUM_LAYERS):
                last = layer == NUM_LAYERS - 1
                h_cur = dense_layer(whh_t, h_cur, HK, layer + 1,
                                    out_f32r=(READOUT_F32R and last))
                pe_warmup(40 if last else 20, h_shards[-1])

            # ---- readout: out = h @ Why (b_y added on host) ----
            n_halves = V_SH // NC_CHUNK          # 2
            n_sub = NC_CHUNK // NSUB             # 4
            with tc.tile_pool(name="outsb", bufs=4) as out_pool, \
                 tc.tile_pool(name="rpsum", bufs=4, space="PSUM") as rpsum:
                h_r = h_cur
                for half in range(n_halves):
                    why_tiles = why_half0 if half == 0 else why_dma(1)

                    ps_list = [rpsum.tile([B, NSUB], f32, tag="rp",
                                          name=f"rp{half}_{c}")
                               for c in range(n_sub)]
                    for k in range(HK):
                        for c in range(n_sub):
                            nc.tensor.matmul(
                                ps_list[c][:, :],
                                h_r[:, k * B:(k + 1) * B],
                                why_tiles[k][:, c * NSUB:(c + 1) * NSUB],
                                start=(k == 0), stop=(k == HK - 1))
                    for c in range(n_sub):
                        ot = out_pool.tile([B, NSUB], f32, tag="o",
                                           name=f"o{half}_{c}")
                        nc.vector.tensor_copy(ot[:], ps_list[c][:])
                        off = half * NC_CHUNK + c * NSUB
                        nc.gpsimd.dma_start(out=out_d[:, off:off + NSUB], in_=ot[:])
            why_pool.release()

    nc.compile()
    return nc


def _get_nc():
    if "nc" not in _cache:
        _ensure_axon_hooks()
        _cache["nc"] = _build_nc()
    return _cache["nc"]


def make_in_maps(X, emb, W_xh, W_hh, W_hy, b_h, b_y):
    X = np.asarray(X)
    emb = np.asarray(emb, dtype=np.float32)
    W_xh = np.asarray(W_xh, dtype=np.float32)
    W_hh = np.asarray(W_hh, dtype=np.float32)
    W_hy = np.asarray(W_hy, dtype=np.float32)
    b_h = np.asarray(b_h, dtype=np.float32)

    idx = np.asarray(X[:, -1], dtype=np.int64)
    x = emb[idx]                                    # [B, E]
    # xt_tiled[p, k*B + b] = x[b, k*128 + p]
    xt = np.ascontiguousarray(
        x.reshape(B, EK, 128).transpose(2, 1, 0).reshape(128, EK * B))

    in_maps = []
    for c in range(N_CORES):
        sl = slice(c * H_SH, (c + 1) * H_SH)
        in_maps.append({
            "xt": xt,
            "wxh": np.ascontiguousarray(W_xh[:, sl]),
            "whh": np.ascontiguousarray(W_hh[:, sl]),
            "why": np.ascontiguousarray(W_hy[:, c * V_SH:(c + 1) * V_SH]),
            # bh_sh[p, i] = b_h[c*256 + i*128 + p]
            "bh": np.ascontiguousarray(b_h[sl].reshape(ML, 128).T),
        })
    return in_maps


def kernel(X, emb, W_xh, W_hh, W_hy, b_h, b_y):
    from concourse.bass_utils import run_bass_kernel_spmd

    nc = _get_nc()
    in_maps = make_in_maps(X, emb, W_xh, W_hh, W_hy, b_h, b_y)
    res = run_bass_kernel_spmd(nc, in_maps, core_ids=list(range(N_CORES)))
    out = np.concatenate([res.results[c]["out"] for c in range(N_CORES)], axis=1)
    out = out + np.asarray(b_y, dtype=np.float32)[None, :]
    return out.astype(np.float32)


# revision 24
# speedup vs baseline: 1.1887x; 1.1205x over previous
"""Trainium2 Bass kernel for nn_RNN_87351044866483.

Reference semantics: every timestep is independent (no recurrence carried
across t) and only the last timestep feeds the readout, so the output
depends only on X[:, -1]:

    x   = emb[X[:, -1]]                  # [B, E]
    h   = tanh(x @ W_xh + b_h)           # [B, H]
    h   = tanh(h @ W_hh + b_h)   (x4)
    out = h @ W_hy + b_y                 # [B, V]

Sharding (8 cores):
  * h-chain is tensor-parallel: W_xh / W_hh / b_h are column-sharded
    (256 features per core); each core computes its 2 m-tiles of the next
    h and an AllGather rebuilds the full h between layers. This matters
    because with B=64 the PE is LDWEIGHTS-bound (fp32 weights load at
    ~426ns per 128x128 tile vs ~107ns of streaming), so replicating the
    h-chain costs 8x the weight-load time.
  * W_hy / b_y are column-sharded (V/8 = 4000 per core); each core emits
    its own output slice, gathered on the host.

On-chip layout: activations are feature-major ("hT layout"):
hT_sb[p, m*64 + b] = h[b, m*128 + p]. Core c owns m-tiles {2c, 2c+1}, so
rank-order AllGather output is exactly the next hT layout (512B segments).
The h-chain stays full fp32 (the tanh layers amplify injected error ~5x
per layer); the readout uses float32r single-pass matmuls (error lands
below the fp32 summation-order noise floor of this problem).
"""

import sys
import types

import numpy as np

B = 64
E = 1024
H = 2048
V = 32000
N_CORES = 8
V_SH = V // N_CORES       # 4000
H_SH = H // N_CORES       # 256 features per core
ML = H_SH // 128          # 2 local m-tiles
NUM_LAYERS = 4

EK = E // 128   # 8  k-tiles in layer 0
HK = H // 128   # 16 k-tiles in hidden layers
HM = H // 128   # 16 m-tiles of hidden features
NC_CHUNK = 2000           # W_hy columns per streamed tile (1 MB DMA)
NSUB = 500                # readout psum free dim (<=512 fp32)
WHY_BUFS = 16             # one full half of W_hy resident (prefetched)

READOUT_F32R = True

_cache = {}


def _ensure_axon_hooks():
    """antenv.axon_hooks is absent in this image; recreate it so
    run_bass_kernel_spmd(trace=True) can import it (used by test.py)."""
    if "antenv.axon_hooks" in sys.modules:
        return
    m = types.ModuleType("antenv.axon_hooks")
    m._hook = None
    m.set_axon_ntff_profile_hook = lambda h: setattr(m, "_hook", h)
    m.get_axon_ntff_profile_hook = lambda: m._hook
    sys.modules["antenv.axon_hooks"] = m
    try:
        import antenv
        antenv.axon_hooks = m
        from trn_agent_boot.trn_boot import _ntff_profile_via_ctypes
        hook = _ntff_profile_via_ctypes("/opt/axon/libaxon_pjrt.so")
        m.set_axon_ntff_profile_hook(hook)
    except Exception:
        pass


def _build_nc():
    import concourse.bacc as bacc
    import concourse.mybir as mybir
    import concourse.tile as tile
    from concourse.tile import add_dep_helper

    f32 = mybir.dt.float32
    f32r = mybir.dt.float32r
    Tanh = mybir.ActivationFunctionType.Tanh

    nc = bacc.Bacc(None, target_bir_lowering=False, num_devices=N_CORES)

    xt_d = nc.dram_tensor("xt", [128, EK * B], f32, kind="ExternalInput")
    # Weight shards arrive host-repacked in the SBUF layout
    # [p, k*H_SH + j] = W[k*128 + p, j]: the DMA is contiguous per
    # partition (strided repack on SWDGE costs ~25ns/segment of Q7
    # descriptor build and stalls the queue for tens of us).
    wxh_d = nc.dram_tensor("wxh", [128, EK * H_SH], f32, kind="ExternalInput")
    whh_d = nc.dram_tensor("whh", [128, HK * H_SH], f32, kind="ExternalInput")
    why_d = nc.dram_tensor("why", [H, V_SH],
                           f32r if READOUT_F32R else f32, kind="ExternalInput")
    bh_d = nc.dram_tensor("bh", [128, ML], f32, kind="ExternalInput")
    out_d = nc.dram_tensor("out", [B, V_SH], f32, kind="ExternalOutput")

    # Per-layer collective staging buffers (DRAM). Shared address space is
    # required for the AllGather output.
    n_dense = 1 + NUM_LAYERS
    cc_in = [nc.dram_tensor(f"cc_in{l}", [128, H_SH // 2], f32)
             for l in range(n_dense)]
    cc_out = [nc.dram_tensor(f"cc_out{l}", [N_CORES, 128, H_SH // 2], f32,
                             addr_space="Shared")
              for l in range(n_dense)]

    groups = [list(range(N_CORES))]

    with tile.TileContext(nc) as tc:
        with tc.tile_pool(name="const", bufs=1) as const_pool, \
             tc.tile_pool(name="wsb", bufs=1) as w_pool, \
             tc.tile_pool(name="hg", bufs=2) as hg_pool, \
             tc.tile_pool(name="hs", bufs=2) as hs_pool, \
             tc.tile_pool(name="lpsum", bufs=2, space="PSUM") as lpsum:

            bh_t = const_pool.tile([128, ML], f32, tag="bh")
            nc.sync.dma_start(out=bh_t[:], in_=bh_d[:])
            xt_t = const_pool.tile([128, EK * B], f32, tag="xt")
            nc.sync.dma_start(out=xt_t[:], in_=xt_d[:])

            # Column-sharded weights, repacked so k-tile k / local column j
            # sits at sbuf[:, k*H_SH + j] (single large DMA each).
            wxh_t = w_pool.tile([128, EK * H_SH], f32, tag="wxh")
            nc.gpsimd.dma_start(out=wxh_t[:], in_=wxh_d[:])
            whh_t = w_pool.tile([128, HK * H_SH], f32, tag="whh")
            whh_dma = nc.gpsimd.dma_start(out=whh_t[:], in_=whh_d[:])

            mm_dt = f32r if READOUT_F32R else f32
            why_pool = tc.alloc_tile_pool(name="why", bufs=WHY_BUFS)

            def why_dma(half):
                tiles = []
                for k in range(HK):
                    wt = why_pool.tile([128, NC_CHUNK], mm_dt, tag="why",
                                       name=f"why{half}_{k}")
                    d = nc.sync.dma_start(
                        out=wt[:],
                        in_=why_d[k * 128:(k + 1) * 128,
                                  half * NC_CHUNK:(half + 1) * NC_CHUNK])
                    if half == 0 and k == 0:
                        add_dep_helper(d.ins, whh_dma.ins, sync=True,
                                       reason="why stream after h-chain weights")
                    tiles.append(wt)
                return tiles

            # Prefetch half 0 of W_hy on the sync queue (nothing is emitted
            # behind it there, so ring-full stalls are harmless); the first
            # transfer waits for the h-chain weights so layer 0 is not
            # bandwidth-starved at startup.
            why_half0 = why_dma(0)

            # PE warm-up: junk matmuls into a scratch psum bank keep the HAM
            # clock at 2.4 GHz across the AllGather gaps (PE-idle > ~3.4us
            # re-throttles to 1.2 GHz, doubling every matmul afterwards).
            # Each batch reads the layer's h_shard so the scheduler cannot
            # hoist it ahead of that layer (hoisted junk serializes in front
            # of real work on the PE).
            warm_ps = lpsum.tile([128, B], f32, tag="warm", name="warm_ps")

            def pe_warmup(n, dep_t):
                for w in range(n):
                    nc.tensor.matmul(
                        warm_ps[:, :], wxh_t[:, 0:128], dep_t[:, 0:B],
                        start=True, stop=True)

            def dense_layer(w_t, rhs_t, nk, lidx):
                """Local m-shard matmul + tanh, then AllGather into the full
                hT layout. Returns the gathered [128, HM*B] tile."""
                ps = lpsum.tile([128, ML * B], f32, tag="lp", name=f"lp{lidx}")
                h_shard = hs_pool.tile([128, ML * B], f32, tag="hs",
                                       name=f"hs{lidx}")
                h_shards.append(h_shard)
                for k in range(nk):
                    for i in range(ML):
                        nc.tensor.matmul(
                            ps[:, i * B:(i + 1) * B],
                            w_t[:, k * H_SH + i * 128: k * H_SH + (i + 1) * 128],
                            rhs_t[:, k * B:(k + 1) * B],
                            start=(k == 0 and i == 0),
                            stop=(k == nk - 1 and i == ML - 1))
                for i in range(ML):
                    nc.scalar.activation(
                        h_shard[:, i * B:(i + 1) * B],
                        ps[:, i * B:(i + 1) * B], Tanh,
                        bias=bh_t[:, i:i + 1])
                nc.scalar.dma_start(out=cc_in[lidx][:], in_=h_shard[:])
                nc.gpsimd.collective_compute(
                    "AllGather", mybir.AluOpType.bypass,
                    replica_groups=groups,
                    ins=[cc_in[lidx][:]],
                    outs=[cc_out[lidx][:]])
                h_full = hg_pool.tile([128, HM * B], f32, tag="hg",
                                      name=f"hf{lidx}")
                nc.scalar.dma_start(
                    out=h_full[:].rearrange("p (c q) -> p c q", c=N_CORES),
                    in_=cc_out[lidx].rearrange("c p q -> p c q"))
                return h_full

            h_shards = []
            h_cur = dense_layer(wxh_t, xt_t, EK, 0)
            pe_warmup(20, h_shards[-1])
            for layer in range(NUM_LAYERS):
                h_cur = dense_layer(whh_t, h_cur, HK, layer + 1)
                pe_warmup(20, h_shards[-1])

            # ---- readout: out = h @ Why (b_y added on host) ----
            n_halves = V_SH // NC_CHUNK          # 2
            n_sub = NC_CHUNK // NSUB             # 4
            with tc.tile_pool(name="hr", bufs=1) as hr_pool, \
                 tc.tile_pool(name="outsb", bufs=4) as out_pool, \
                 tc.tile_pool(name="rpsum", bufs=4, space="PSUM") as rpsum:
                if READOUT_F32R:
                    h_r = hr_pool.tile([128, HM * B], f32r, tag="hr", name="h_r")
                    nc.vector.tensor_copy(h_r[:], h_cur[:])
                else:
                    h_r = h_cur
                for half in range(n_halves):
                    why_tiles = why_half0 if half == 0 else why_dma(1)

                    ps_list = [rpsum.tile([B, NSUB], f32, tag="rp",
                                          name=f"rp{half}_{c}")
                               for c in range(n_sub)]
                    for k in range(HK):
                        for c in range(n_sub):
                            nc.tensor.matmul(
                                ps_list[c][:, :],
                                h_r[:, k * B:(k + 1) * B],
                                why_tiles[k][:, c * NSUB:(c + 1) * NSUB],
                                start=(k == 0), stop=(k == HK - 1))
                    for c in range(n_sub):
                        ot = out_pool.tile([B, NSUB], f32, tag="o",
                                           name=f"o{half}_{c}")
                        nc.vector.tensor_copy(ot[:], ps_list[c][:])
                        off = half * NC_CHUNK + c * NSUB
                        nc.gpsimd.dma_start(out=out_d[:, off:off + NSUB], in_=ot[:])
            why_pool.release()

    nc.compile()
    return nc


def _get_nc():
    if "nc" not in _cache:
        _ensure_axon_hooks()
        _cache["nc"] = _build_nc()
    return _cache["nc"]


def make_in_maps(X, emb, W_xh, W_hh, W_hy, b_h, b_y):
    X = np.asarray(X)
    emb = np.asarray(emb, dtype=np.float32)
    W_xh = np.asarray(W_xh, dtype=np.float32)
    W_hh = np.asarray(W_hh, dtype=np.float32)
    W_hy = np.asarray(W_hy, dtype=np.float32)
    b_h = np.asarray(b_h, dtype=np.float32)

    idx = np.asarray(X[:, -1], dtype=np.int64)
    x = emb[idx]                                    # [B, E]
    # xt_tiled[p, k*B + b] = x[b, k*128 + p]
    xt = np.ascontiguousarray(
        x.reshape(B, EK, 128).transpose(2, 1, 0).reshape(128, EK * B))

    in_maps = []
    for c in range(N_CORES):
        sl = slice(c * H_SH, (c + 1) * H_SH)
        in_maps.append({
            "xt": xt,
            "wxh": np.ascontiguousarray(
                W_xh[:, sl].reshape(EK, 128, H_SH).transpose(1, 0, 2)
                .reshape(128, EK * H_SH)),
            "whh": np.ascontiguousarray(
                W_hh[:, sl].reshape(HK, 128, H_SH).transpose(1, 0, 2)
                .reshape(128, HK * H_SH)),
            "why": np.ascontiguousarray(W_hy[:, c * V_SH:(c + 1) * V_SH]),
            # bh_sh[p, i] = b_h[c*256 + i*128 + p]
            "bh": np.ascontiguousarray(b_h[sl].reshape(ML, 128).T),
        })
    return in_maps


def kernel(X, emb, W_xh, W_hh, W_hy, b_h, b_y):
    from concourse.bass_utils import run_bass_kernel_spmd

    nc = _get_nc()
    in_maps = make_in_maps(X, emb, W_xh, W_hh, W_hy, b_h, b_y)
    res = run_bass_kernel_spmd(nc, in_maps, core_ids=list(range(N_CORES)))
    out = np.concatenate([res.results[c]["out"] for c in range(N_CORES)], axis=1)
    out = out + np.asarray(b_y, dtype=np.float32)[None, :]
    return out.astype(np.float32)
